# revision 71
# baseline (speedup 1.0000x reference)
"""Distillation loss (CE + top-k combo KLs + rNTK KL) on 8 Trainium2 cores.

The reference's additive -1000 masks exactly restrict each softmax to its
unmasked entries, so the loss decomposes into per-row scalars:

  Zce = sum_v exp(s_v)       Zs4 = sum_v exp(s_v/4)     Zt4 = sum_v exp(t_v/4)
  Gt  = sum_v exp(t_v/4)*t_v Gs  = sum_v exp(t_v/4)*s_v (G = Gt - Gs)
  top-3 of s per row

Device (data-parallel over the batch, 256 rows/core): a single fp8 stream
holds both logit matrices transposed (vocab on the partition axis) in an
interleaved [t_h|s_h|1] layout built on the host.  Per vocab-tile chunk:

  ACT : exp(x/4) for ~half the tiles of both matrices (fp8 out)
  DVE : exp(x/4) for the other half and exp(s) for Zce via the Schraudolph
        bit-trick (one fused tensor_scalar each: fp8 bits = round(A*x+B),
        uint8 saturation = underflow clamp).  The same ACT/DVE split is used
        for student and teacher, so the identical approximation bias cancels
        in the KL's log(Zs4)-log(Zt4); only the scale-invariant ratio G/Zt4
        and the tiny CE shift remain exposed.
  PE  : all vocab reductions as fp8 DoubleRow matmuls (K=256 vocab-tile
        pairs) accumulating in PSUM over all 250 tiles: diag(et^T t) = Gt,
        diag(et^T s) = Gs, a ones-column in the moving stream gives Zt4,
        and a ones-pair stationary over [es4|es1] tiles gives Zs4/Zce.
        The [es4|es1] matmuls run one chunk late so the PE never waits on
        the DVE exp chain.

Host (float64 epilogue): exact top-3 of the original fp32 student
(argpartition), teacher/student gathers, the 3-term rNTK corrections, 4
tiny combo KLs, and the final scalar.  Tolerance is 2e-2 relative; the fp8
streaming + bit-trick exp land at ~1.7e-3.
"""

import sys

import numpy as np
import ml_dtypes

try:
    import concourse.bass as bass
except ImportError:  # pragma: no cover
    sys.path.insert(0, "/opt/trn_rl_repo")
    import concourse.bass as bass

import concourse.bacc as bacc
import concourse.mybir as mybir
from concourse.bass_utils import run_bass_kernel_spmd
from concourse.tile import TileContext

# Problem shape (hardcoded per spec).
B, V = 2048, 32000
NCORES = 8
RPC = B // NCORES          # rows per core = 256
P = 128                    # partitions
NT = RPC // P              # row tiles per core = 2
W = 4000                   # row-major chunk width for max8
NCH = V // W               # chunks per row = 8
K = 3
TEMP = 4.0
GAMMA = 0.05

# transposed stream geometry
NVT = V // P               # vocab tiles = 250
# chunk sizes (vocab tiles, even for DoubleRow pairing): small first chunks
# for a fast pipeline ramp, large ones to amortize ACT instruction overhead
CHUNKS = [2, 4, 8] + [20] * 11 + [8, 8]

# Schraudolph bit-trick exp constants: exp(x*s) ~= bitcast_fp8(round(A*x+B)).
# The temp-4 variant runs on the last ~1/4 of each chunk's vocab tiles (same
# fraction for student and teacher, so the distribution-identical bias
# cancels in the KL log-ratio); the temp-1 variant (Zce) runs on all tiles.
# uint8 output saturation clamps exp underflow to fp8 zero.
A8 = 8.0 / (4.0 * np.log(2.0))            # fp8 e4m3, exp(x/4)
B8 = 7 * 8 - 0.043 * 8
A18 = 8.0 / np.log(2.0)                   # fp8 e4m3, exp(x)
B18 = B8
HBW = 264                  # half block: [t(128)|s(128)|one|pad(7)]
TSW = 2 * HBW              # 528 cols per vocab tile; 528B stride %16==0

F32 = mybir.dt.float32
BF16 = mybir.dt.bfloat16
FP8 = mybir.dt.float8e4
U32 = mybir.dt.uint32
NP_BF16 = ml_dtypes.bfloat16
NP_FP8 = ml_dtypes.float8_e4m3

_NC = None


def _build_bass():
    global _NC
    if _NC is not None:
        return _NC

    nc = bacc.Bacc("TRN2", target_bir_lowering=False)

    ts_d = nc.dram_tensor("ts", [P, NVT * TSW], FP8, kind="ExternalInput")
    id_d = nc.dram_tensor("ident", [P, 129], BF16, kind="ExternalInput")
    zq_d = nc.dram_tensor("zq_out", [1, 512], F32, kind="ExternalOutput")
    gs_d = nc.dram_tensor("gstats", [P, 6], F32, kind="ExternalOutput")

    EXP = mybir.ActivationFunctionType.Exp
    MUL = mybir.AluOpType.mult
    ADD = mybir.AluOpType.add

    with TileContext(nc) as tc:
        with (
            tc.tile_pool(name="work", bufs=1) as work_pool,
            tc.psum_pool(name="ps", bufs=1) as ps_pool,
        ):
            # warm the exp table before any data arrives (no DMA dependency)
            warm = work_pool.tile([P, 1], BF16, tag="warm", bufs=1)
            nc.vector.memset(warm[:], 0.0)
            nc.scalar.activation(out=warm[:], in_=warm[:],
                                 func=EXP, scale=1.0)
            # fp8 ones pair: DoubleRow stationary for the Zs4/Zce matmuls
            # (padded so the row-set step is 16B, per s3_lw dual-fp8 rules)
            ones8 = work_pool.tile([P, 32], FP8, tag="ones8", bufs=1)
            nc.vector.memset(ones8[:], 1.0)
            ones8_v = ones8.rearrange("p (k o) -> p k o", k=2, o=16)[:, :, 0:1]
            # ident feeds only the final diag extraction; its DMA is emitted
            # after the first ts chunks so it never delays the ramp
            ident = work_pool.tile([P, 129], BF16, tag="ident", bufs=1)

            g_ps = [ps_pool.tile([P, 257], F32, tag=f"g{h}", name=f"g_ps{h}")
                    for h in range(2)]
            zq_ps = ps_pool.tile([1, 512], F32)

            pending_zq = []

            def flush_zq():
                for esq_prev, u, st, sp in pending_zq:
                    nc.tensor.matmul(out=zq_ps[:], lhsT=ones8_v[:],
                                     rhs=esq_prev[:, 2 * u:2 * u + 2],
                                     start=st, stop=sp,
                                     perf_mode=mybir.MatmulPerfMode.DoubleRow)
                pending_zq.clear()

            DR = mybir.MatmulPerfMode.DoubleRow
            MAXC = max(CHUNKS)
            col0 = 0
            tile0 = 0
            for ch, CHT in enumerate(CHUNKS):
                ts_t = work_pool.tile([P, MAXC * TSW], FP8, tag="ts", bufs=3)
                nc.sync.dma_start(
                    out=ts_t[:, 0:CHT * TSW],
                    in_=ts_d[:, col0:col0 + CHT * TSW])
                ts_v = ts_t[:, 0:CHT * TSW].rearrange(
                    "p (t h j) -> p t h j", t=CHT, h=2, j=HBW)

                et_t = work_pool.tile([P, MAXC * 256], FP8, tag="et", bufs=3)
                et_v = et_t[:, 0:CHT * 256].rearrange(
                    "p (t h j) -> p t h j", t=CHT, h=2, j=128)
                et_p = et_t[:, 0:CHT * 256].rearrange(
                    "p (t c) -> p t c", t=CHT, c=256)
                # esq: [p, tile, {es4|es1}, half, col] (512B/tile, DR-packable)
                esq_t = work_pool.tile([P, MAXC, 2, 2, 128], FP8, tag="esq",
                                       bufs=3)

                # ACT computes exp(x/4) on the first S tiles of both sides;
                # the last NS tiles use the DVE bit-trick exp instead
                NS = 2 * (CHT // 4)   # even: odd tile counts break DVE 2x
                S = CHT - NS
                nc.scalar.activation(out=esq_t[:, 0:S, 0],
                                     in_=ts_v[:, 0:S, :, 128:256],
                                     func=EXP, scale=0.25)
                nc.scalar.activation(out=et_v[:, 0:S],
                                     in_=ts_v[:, 0:S, :, 0:128],
                                     func=EXP, scale=0.25)
                if NS:
                    nc.vector.tensor_scalar(
                        out=esq_t[:, S:CHT, 0].bitcast(mybir.dt.uint8),
                        in0=ts_v[:, S:CHT, :, 128:256],
                        scalar1=float(A8), scalar2=float(B8),
                        op0=MUL, op1=ADD)
                    nc.vector.tensor_scalar(
                        out=et_v[:, S:CHT].bitcast(mybir.dt.uint8),
                        in0=ts_v[:, S:CHT, :, 0:128],
                        scalar1=float(A8), scalar2=float(B8),
                        op0=MUL, op1=ADD)
                # es1 = exp(s) for Zce: bit-trick on all tiles
                nc.vector.tensor_scalar(
                    out=esq_t[:, 0:CHT, 1].bitcast(mybir.dt.uint8),
                    in0=ts_v[:, 0:CHT, :, 128:256],
                    scalar1=float(A18), scalar2=float(B18),
                    op0=MUL, op1=ADD)

                # Zs4/Zce matmuls run one chunk late so the PE never waits
                # on the DVE exp chain
                flush_zq()
                # G matmuls: fp8 DoubleRow contracts vocab-tile PAIRS (K=256)
                for u in range(CHT // 2):
                    pr = tile0 // 2 + u
                    st = (pr == 0)
                    sp = (pr == NVT // 2 - 1)
                    for h in range(2):
                        nc.tensor.matmul(
                            out=g_ps[h][:],
                            lhsT=et_p[:, 2 * u:2 * u + 2, h * 128:h * 128 + 128],
                            rhs=ts_v[:, 2 * u:2 * u + 2, h, 0:257],
                            start=st, stop=sp, perf_mode=DR)
                    pending_zq.append((esq_t, u, pr == 0, pr == NVT // 2 - 1))
                col0 += CHT * TSW
                tile0 += CHT
                if ch == 0:
                    nc.sync.dma_start(out=ident[:], in_=id_d[:, :])
            flush_zq()

            # --- extraction ---
            gstat = work_pool.tile([P, 6], F32, tag="gstat", bufs=1)
            scrap = work_pool.tile([P, 128], BF16, tag="scrap", bufs=1)
            for h in range(2):
                nc.vector.scalar_tensor_tensor(
                    out=scrap[:], in0=g_ps[h][:, 0:128], scalar=1.0,
                    in1=ident[:, 0:128], op0=MUL, op1=MUL,
                    accum_out=gstat[:, 3 * h + 0:3 * h + 1])
                nc.vector.scalar_tensor_tensor(
                    out=scrap[:], in0=g_ps[h][:, 128:256], scalar=1.0,
                    in1=ident[:, 0:128], op0=MUL, op1=MUL,
                    accum_out=gstat[:, 3 * h + 1:3 * h + 2])
                nc.vector.tensor_copy(out=gstat[:, 3 * h + 2:3 * h + 3],
                                      in_=g_ps[h][:, 256:257])
            zq_sb = work_pool.tile([1, 512], F32, tag="zq", bufs=1)
            nc.vector.tensor_copy(out=zq_sb[:], in_=zq_ps[:])
            nc.sync.dma_start(out=gs_d[:, :], in_=gstat[:])
            nc.sync.dma_start(out=zq_d[:, :], in_=zq_sb[:])

    if not nc.is_finalized():
        nc.finalize()
    _NC = nc
    return nc


def _prep_core_inputs(student, teacher):
    """student/teacher: fp32 [B, V].  Returns per-core input maps."""
    s8 = student.astype(NP_FP8)
    t8 = teacher.astype(NP_FP8)

    ident = np.zeros((P, 129), dtype=NP_BF16)
    ident[np.arange(P), np.arange(P)] = 1.0
    ident[:, 128] = 1.0

    in_maps = []
    for c in range(NCORES):
        r0 = c * RPC
        # [v, p, h, j] = x[h*128+j, v*128+p]  (vocab tile v, partition p,
        # row-half h, row-in-half j)
        tt8 = np.ascontiguousarray(t8[r0:r0 + RPC]).T.reshape(NVT, P, 2, 128)
        ss8 = np.ascontiguousarray(s8[r0:r0 + RPC]).T.reshape(NVT, P, 2, 128)
        ts = np.zeros((P, NVT, 2, HBW), dtype=NP_FP8)
        ts[:, :, :, 0:128] = tt8.transpose(1, 0, 2, 3)
        ts[:, :, :, 128:256] = ss8.transpose(1, 0, 2, 3)
        ts[:, :, :, 256] = np.float32(1.0)
        in_maps.append({
            "ts": ts.reshape(P, NVT * TSW),
            "ident": ident,
        })
    return in_maps


def _run_device(student, teacher, trace=False, **kw):
    nc = _build_bass()
    student = np.asarray(student, dtype=np.float32)
    teacher = np.asarray(teacher, dtype=np.float32)
    in_maps = _prep_core_inputs(student, teacher)
    bkr = run_bass_kernel_spmd(nc, in_maps, core_ids=list(range(NCORES)),
                               trace=trace, **kw)
    return bkr


def _adw(i, j):
    t, tp = i + 1, j + 1
    return 1.0 / (1.5 + abs(t - tp)) * 2.0 * float(np.exp(-GAMMA * (t + tp)))


def _recover_top3(student):
    """Exact fp32 top-3 values+indices per row."""
    i3 = np.argpartition(-student, K - 1, axis=1)[:, :K]
    v3 = np.take_along_axis(student, i3, axis=1)
    o3 = np.argsort(-v3, axis=1, kind="stable")
    gidx = np.take_along_axis(i3, o3, axis=1)
    vals = np.take_along_axis(v3, o3, axis=1)
    return vals.astype(np.float64), gidx.astype(np.int64)


def _finalize(student, teacher, target, results):
    """Host epilogue in float64."""
    zce = np.empty((B,), np.float64)
    zs4 = np.empty((B,), np.float64)
    zt4 = np.empty((B,), np.float64)
    g = np.empty((B,), np.float64)

    for c in range(NCORES):
        out = results[c]
        zq = out["zq_out"].reshape(512).astype(np.float64)
        gst = out["gstats"].reshape(P, 6).astype(np.float64)
        for h in range(2):
            r = slice(c * RPC + h * P, c * RPC + (h + 1) * P)
            zs4[r] = zq[h * 128:(h + 1) * 128]
            zce[r] = zq[256 + h * 128:256 + (h + 1) * 128]
            g[r] = gst[:, 3 * h + 0] - gst[:, 3 * h + 1]
            zt4[r] = gst[:, 3 * h + 2]

    sv, si = _recover_top3(student)

    tgt = np.asarray(target).astype(np.int64).reshape(B)
    s_t = np.take_along_axis(student, tgt[:, None], axis=1)[:, 0].astype(np.float64)
    tv = np.take_along_axis(teacher, si, axis=1).astype(np.float64)

    # CE (mean reduction)
    loss_ce = float(np.mean(np.log(zce) - s_t))

    # combo KLs over restricted softmaxes
    def restricted_kl(cols):
        a = tv[:, cols] / TEMP
        bq = sv[:, cols] / TEMP
        lse_a = np.log(np.sum(np.exp(a), axis=1, keepdims=True))
        lse_b = np.log(np.sum(np.exp(bq), axis=1, keepdims=True))
        lp = a - lse_a
        lq = bq - lse_b
        p = np.exp(lp)
        return np.sum(p * (lp - lq))  # sum over rows and entries

    combos = [(0, 1), (0, 2), (1, 2), (0, 1, 2)]
    total = 0.0
    for comb in combos:
        w = _adw(comb[0], comb[1]) if len(comb) == 2 else 1.0
        total += w * restricted_kl(list(comb)) * (TEMP ** 2) / B
    loss_kd = total / len(combos)

    # rNTK: complement-of-top3 KL via corrected full sums
    e_sv = np.exp(sv / TEMP)
    e_tv = np.exp(tv / TEMP)
    zsm = zs4 - e_sv.sum(1)
    ztm = zt4 - e_tv.sum(1)
    gm = g - np.sum(e_tv * (tv - sv), axis=1)
    kl_rntk = gm / (TEMP * ztm) - np.log(ztm) + np.log(zsm)
    not_loss_kd = float(np.sum(kl_rntk)) * (TEMP ** 2) / B

    return np.float32(loss_ce + loss_kd + not_loss_kd)


def kernel(logits_student, logits_teacher, target):
    student = np.ascontiguousarray(np.asarray(logits_student, dtype=np.float32))
    teacher = np.ascontiguousarray(np.asarray(logits_teacher, dtype=np.float32))
    bkr = _run_device(student, teacher, trace=False)
    return _finalize(student, teacher, target, bkr.results)
